# revision 1
# baseline (speedup 1.0000x reference)
"""Trainium2 Bass kernel for nn_NetAtom (Behler-Parrinello segment reduce).

Full-input contract: kernel(**inputs) takes the complete (unsharded) numpy
arrays from setup_inputs() and returns the full [2K] output.

Strategy (8 cores, atom sharding):
  - Host: shard atoms across the 8 cores (padded to 12800/core; padded logic
    rows are zero so padded atoms contribute nothing), pre-transpose desc to
    [D, n] bf16, and pre-pack logic.T into the exact per-partition SBUF
    stream layout [128, n_chunks * 4 * K] fp8 so every logic DMA is a large
    fully-contiguous transfer.
  - Device (per core, bf16 MLP matmuls + fp8 matvec, fp32 PSUM accumulation):
      h1T = tanh(W1 @ descT + b1)          [256, n]  (2 partition tiles)
      h2T = tanh(W2 @ h1T + b2)            [256, n]
      per 128-atom subchunk j:  pv[n,2] = h2T_j.T @ W3T
      v[:,0] = pv[:,0] + b3[0]   (DVE)
      v[:,1] = softplus(pv[:,1] + b3[1]) = Ln(Exp(.) + 1)  (ACT)
      psum[2,500] (x2 banks) += v_j.T @ logicT_j   accumulated over all
      subchunks of both species.
  - The Ln lives in a different ACT function set than Tanh/Exp, and each set
    switch costs a ~1.3us table load; chunks are processed in groups of G=8
    with one Ln per group (logic tiles stay resident until their matvec).
  - 3-stage software pipeline (A: loads+L1, B: L2, C: L3+v) so the PE never
    waits on the same chunk's tanh; the group matvec is spread over
    subsequent slots.
  - Host: sum the 8 per-core [2,1000] partials, concat -> [2000].
"""

import contextlib
from collections import deque

import numpy as np
import ml_dtypes

import concourse.mybir as mybir
import concourse.tile as tile
from concourse import bacc
from concourse.bass_utils import run_bass_kernel_spmd

BF = mybir.dt.bfloat16
F8 = mybir.dt.float8e4
F32 = mybir.dt.float32
ACTF = mybir.ActivationFunctionType

D = 128        # descriptor size
H = 256        # hidden width
N = 100000     # atoms per species (full)
K = 1000       # structures
NCORES = 8
CHUNK = 512    # atoms per pipeline chunk
NJ = CHUNK // 128          # 128-atom subchunks per chunk
NCH = 25                   # chunks per core per species
NA = NCH * CHUNK           # 12800 atoms per core (padded); 8*12800 = 102400
KP = 1024                  # padded K stride (16B-aligned j-step)
KH = K // 2                # structure half (one PSUM bank each)
G = 8          # chunks per Ln group
MV_DRAIN = 1   # matvec chunks emitted per pipeline slot
SC = 2         # chunks per logic/desc superchunk DMA

WCOLS = H + 2 * H + 4      # packed weight cols: w1t | w2t | w3t


def build_nc(repeat=None, mode='full'):
    nc = bacc.Bacc()

    ins = {}
    for s in (0, 1):
        ins[f"logicL{s}"] = nc.dram_tensor(f"logicL{s}", [128, NCH * NJ * KP],
                                           F8, kind="ExternalInput")
        ins[f"descT{s}"] = nc.dram_tensor(f"descT{s}", [D, NA], BF,
                                          kind="ExternalInput")
        ins[f"wpack{s}"] = nc.dram_tensor(f"wpack{s}", [128, WCOLS], BF,
                                          kind="ExternalInput")
        ins[f"bpack{s}"] = nc.dram_tensor(f"bpack{s}", [128, 6], F32,
                                          kind="ExternalInput")
    out_d = nc.dram_tensor("out", [2, K], F32, kind="ExternalOutput")

    with tile.TileContext(nc) as tc:
        with tc.tile_pool(name="consts", bufs=1) as consts, \
             tc.tile_pool(name="descp", bufs=3) as descp, \
             tc.tile_pool(name="logicp", bufs=G + 3) as logicp, \
             tc.tile_pool(name="hp", bufs=4) as hp, \
             tc.tile_pool(name="vp", bufs=3) as vp, \
             tc.tile_pool(name="outp", bufs=1) as outp, \
             tc.tile_pool(name="ps_mlp", bufs=5, space="PSUM") as ps_mlp, \
             tc.tile_pool(name="ps_v", bufs=1, space="PSUM") as ps_v, \
             tc.tile_pool(name="ps_mv", bufs=1, space="PSUM") as ps_mv:

            _stack = contextlib.ExitStack()
            if repeat:
                _stack.enter_context(tc.For_i(0, repeat, 1))

            # ---- constants: one packed weight + bias DMA per species ----
            wp, bp = {}, {}
            for s in (0, 1):
                wp[s] = consts.tile([128, WCOLS], BF, name=f"wp_{s}")
                nc.sync.dma_start(out=wp[s], in_=ins[f"wpack{s}"][:, :])
                bp[s] = consts.tile([128, 6], F32, name=f"bp_{s}")
                nc.sync.dma_start(out=bp[s], in_=ins[f"bpack{s}"][:, :])

            def w1(s, ht):           # [128 d, 128 h]
                return wp[s][:, ht * 128:(ht + 1) * 128]

            def w2(s, kk, ht):       # [128 h1, 128 h2]
                return wp[s][:, H + kk * H + ht * 128:
                             H + kk * H + (ht + 1) * 128]

            def w3(s, kk):           # [128 h2, 2]
                return wp[s][:, 3 * H + 2 * kk:3 * H + 2 * kk + 2]

            def bias(s, which, i):   # [128, 1] per-partition
                off = {"b1": 0, "b2": 2, "b3": 4}[which] + i
                return bp[s][:, off:off + 1]

            # ---- matvec accumulators: [2, 500] x2, live for whole kernel ----
            pmv = [ps_mv.tile([2, KH], F32, name=f"pmv{h}") for h in (0, 1)]

            # chunk descriptors: (species, chunk index within species)
            chunks = [(s, c) for s in (0, 1) for c in range(NCH)]
            n_chunks = len(chunks)
            mv_emitted = [0]
            last_mv = [None]
            super_state = {}

            def stage_a(cdesc):
                """Superchunk DMA loads + layer 1 + tanh(h1)."""
                s, c = cdesc
                if c % SC == 0:
                    nsc = min(SC, NCH - c)
                    dt = descp.tile([D, SC * CHUNK], BF, name="dt", tag="dt")
                    nc.gpsimd.dma_start(
                        out=dt[:, :nsc * CHUNK],
                        in_=ins[f"descT{s}"][:, c * CHUNK:(c + nsc) * CHUNK])
                    lt = logicp.tile([128, SC * NJ, KP], F8, name="lt",
                                     tag="lt")
                    nc.sync.dma_start(
                        out=lt[:, :nsc * NJ, :],
                        in_=ins[f"logicL{s}"][:, c * NJ * KP:(c + nsc) * NJ * KP]
                            .rearrange("p (j k) -> p j k", k=KP),
                    )
                    super_state["lt"] = lt
                    super_state["dt"] = dt
                off = c % SC
                lt = super_state["lt"][:, off * NJ:(off + 1) * NJ, :]
                dtc = super_state["dt"][:, off * CHUNK:(off + 1) * CHUNK]
                if mode == 'dma':
                    return dict(s=s, lt=lt, h1=None)
                h1 = hp.tile([128, 2, CHUNK], BF, name="h1", tag="h1")
                for ht in (0, 1):
                    p1 = ps_mlp.tile([128, CHUNK], F32, name="pmlp",
                                     tag="pmlp")
                    nc.tensor.matmul(
                        p1[:, :], lhsT=w1(s, ht), rhs=dtc,
                        start=True, stop=True,
                    )
                    nc.scalar.activation(
                        h1[:, ht, :], p1[:, :], ACTF.Tanh,
                        bias=bias(s, "b1", ht), scale=1.0,
                    )
                return dict(s=s, lt=lt, h1=h1)

            def stage_b(meta):
                """Layer 2 + tanh(h2)."""
                s, h1 = meta["s"], meta["h1"]
                h2 = hp.tile([128, 2, CHUNK], BF, name="h2", tag="h2")
                for ht in (0, 1):
                    p2 = ps_mlp.tile([128, CHUNK], F32, name="pmlp",
                                     tag="pmlp")
                    for kk in (0, 1):
                        nc.tensor.matmul(
                            p2[:, :], lhsT=w2(s, kk, ht), rhs=h1[:, kk, :],
                            start=(kk == 0), stop=(kk == 1),
                        )
                    nc.scalar.activation(
                        h2[:, ht, :], p2[:, :], ACTF.Tanh,
                        bias=bias(s, "b2", ht), scale=1.0,
                    )
                meta["h2"] = h2

            def stage_c(meta, grp):
                """Layer 3 + v-even (DVE) + exp stash."""
                s, h2 = meta["s"], meta["h2"]
                pv = ps_v.tile([128, 2 * NJ], F32, name="pv", tag="pv")
                for j in range(NJ):
                    for kk in (0, 1):
                        mm = nc.tensor.matmul(
                            pv[:, 2 * j:2 * j + 2],
                            lhsT=h2[:, kk, j * 128:(j + 1) * 128],
                            rhs=w3(s, kk),
                            start=(kk == 0), stop=(kk == 1),
                        )
                        # keep L3 behind this slot's matvec burst in the PE
                        # stream: its tanh(h2) input lands late, and hoisting
                        # it ahead of ready matvec work stalls the PE.
                        if j == 0 and kk == 0 and last_mv[0] is not None:
                            tile.add_dep_helper(
                                mm.ins, last_mv[0].ins, sync=False,
                                reason="order L3 after matvec burst")

                jj = grp["jj"]
                nc.vector.tensor_scalar_add(
                    grp["vg"][:, jj:jj + NJ, 0],
                    pv[:, 0:2 * NJ:2],
                    bias(s, "b3", 0),
                )
                nc.scalar.activation(
                    grp["tg"][:, jj:jj + NJ], pv[:, 1:2 * NJ:2], ACTF.Exp,
                    bias=bias(s, "b3", 1), scale=1.0,
                )
                meta["vg"] = grp["vg"]
                meta["jj"] = jj
                grp["jj"] = jj + NJ

            def emit_ln(grp):
                gnj = grp["jj"]
                nc.scalar.activation(
                    grp["vg"][:, :gnj, 1], grp["tg"][:, :gnj], ACTF.Ln,
                    bias=1.0, scale=1.0,
                )

            def emit_mv(meta):
                if mode == 'nomv':
                    mv_emitted[0] += 1
                    return
                lt, vg, jj = meta["lt"], meta["vg"], meta["jj"]
                first = mv_emitted[0] == 0
                last = mv_emitted[0] == n_chunks - 1
                for jp in range(0, NJ, 2):
                    for h in (0, 1):
                        last_mv[0] = nc.tensor.matmul(
                            pmv[h][:, :],
                            lhsT=vg[:, jj + jp:jj + jp + 2, 0:2],
                            rhs=lt[:, jp:jp + 2, h * KH:(h + 1) * KH],
                            start=(first and jp == 0),
                            stop=(last and jp == NJ - 2),
                            perf_mode=mybir.MatmulPerfMode.DoubleRow,
                            skip_group_check=True,
                        )
                mv_emitted[0] += 1

            def new_grp():
                return dict(
                    vg=vp.tile([128, G * NJ, 16], F8, name="vg", tag="vg"),
                    tg=vp.tile([128, G * NJ], F32, name="tg", tag="tg"),
                    jj=0, metas=[],
                )

            pending = deque()
            prev_a = None
            prev_b = None
            grp = None
            for ci in range(n_chunks + 2):
                meta = stage_a(chunks[ci]) if ci < n_chunks else None
                if mode == 'dma':
                    continue
                if prev_a is not None:
                    stage_b(prev_a)
                for _ in range(MV_DRAIN):
                    if pending:
                        emit_mv(pending.popleft())
                if prev_b is not None:
                    if grp is None:
                        grp = new_grp()
                    stage_c(prev_b, grp)
                    grp["metas"].append(prev_b)
                    if len(grp["metas"]) == G or prev_a is None:
                        emit_ln(grp)
                        pending.extend(grp["metas"])
                        grp = None
                prev_b = prev_a
                prev_a = meta

            while pending:
                emit_mv(pending.popleft())

            # ---- writeback ----
            osb = outp.tile([2, K], F32, name="osb")
            if mode == 'full':
                for h in (0, 1):
                    nc.vector.tensor_copy(osb[:, h * KH:(h + 1) * KH],
                                          pmv[h][:, :])
            else:
                nc.vector.memset(osb[:, :], 0.0)
            nc.sync.dma_start(out=out_d[:, :], in_=osb[:, :])
            _stack.close()

    nc.compile()
    return nc


_NC_CACHE = None


def _get_nc():
    global _NC_CACHE
    if _NC_CACHE is None:
        _NC_CACHE = build_nc()
    return _NC_CACHE


def make_in_maps(desc0, desc1, logic0, logic1,
                 W1_0, b1_0, W2_0, b2_0, W3_0, b3_0,
                 W1_1, b1_1, W2_1, b2_1, W3_1, b3_1):
    bf16 = ml_dtypes.bfloat16
    fp8 = ml_dtypes.float8_e4m3
    NPAD = NCORES * NA

    per_species = {}
    for s, (desc, logic, W1, b1v, W2, b2v, W3, b3v) in enumerate((
            (desc0, logic0, W1_0, b1_0, W2_0, b2_0, W3_0, b3_0),
            (desc1, logic1, W1_1, b1_1, W2_1, b2_1, W3_1, b3_1))):
        descT = np.zeros((D, NPAD), dtype=bf16)
        descT[:, :N] = np.asarray(desc, np.float32).T.astype(bf16)
        logicT = np.zeros((NPAD, KP), dtype=fp8)
        logicT[:N, :K] = np.asarray(logic, np.float32).T.astype(fp8)
        # SBUF stream layout: [core][128, NCH*NJ*KP], chunk c at cols
        # c*NJ*KP, subchunk j contiguous KP cols, partition = atom % 128.
        logicL = (logicT.reshape(NCORES, NCH, NJ, 128, KP)
                  .transpose(0, 3, 1, 2, 4)
                  .reshape(NCORES, 128, NCH * NJ * KP))
        logicL = np.ascontiguousarray(logicL)

        w1t = np.asarray(W1, np.float32).T                   # [128, 256]
        w2t = (np.asarray(W2, np.float32).T.reshape(2, 128, H)
               .transpose(1, 0, 2).reshape(128, 2 * H))      # [128, 512]
        w3t = (np.asarray(W3, np.float32).T.reshape(2, 128, 2)
               .transpose(1, 0, 2).reshape(128, 4))          # [128, 4]
        wpack = np.concatenate([w1t, w2t, w3t], axis=1).astype(bf16)

        bpack = np.concatenate([
            np.asarray(b1v, np.float32).reshape(2, 128).T,
            np.asarray(b2v, np.float32).reshape(2, 128).T,
            np.broadcast_to(np.asarray(b3v, np.float32), (128, 2)),
        ], axis=1)
        bpack = np.ascontiguousarray(bpack)

        per_species[s] = dict(descT=descT, logicL=logicL,
                              wpack=wpack, bpack=bpack)

    in_maps = []
    for c in range(NCORES):
        m = {}
        for s in (0, 1):
            sp = per_species[s]
            m[f"descT{s}"] = sp["descT"][:, c * NA:(c + 1) * NA]
            m[f"logicL{s}"] = sp["logicL"][c]
            m[f"wpack{s}"] = sp["wpack"]
            m[f"bpack{s}"] = sp["bpack"]
        in_maps.append(m)
    return in_maps


def run(in_maps, trace=False, **kwargs):
    nc = _get_nc()
    return run_bass_kernel_spmd(nc, in_maps, core_ids=list(range(NCORES)),
                                trace=trace, **kwargs)


def kernel(**inputs):
    in_maps = make_in_maps(**inputs)
    res = run(in_maps)
    total = np.zeros((2, K), np.float64)
    for r in res.results:
        total += r["out"].astype(np.float64)
    return np.concatenate([total[0], total[1]]).astype(np.float32)

